# revision 26
# baseline (speedup 1.0000x reference)
"""Routed expert-parallel BruteForce MoE kernel for 8 TRN2 NeuronCores.

Model: N=1024 tokens, D=512 d_model, H=2048 d_hidden, E=8 experts, top-K=2.
  logits = inp @ gate_w.T + gate_b ; top2 -> softmax scores
  y(tok,e) = gelu(x @ w1[e].T + b1[e]) @ w2[e].T + b2[e]
  out = LN( sum_k score_k * y(tok, e_k) )

Strategy: core e owns expert e. The HOST computes the gate logits only to
make the ROUTING decision (which tokens hit expert e); each core receives
just its ~C routed tokens packed [D, C] (C = max per-expert count, padded).
The DEVICE recomputes the gate for its slots in exact f32 (matching the
host's top-2 choice; min 2nd-vs-3rd logit gap is ~2e-4 >> f32 noise) and
derives the softmax score of its own expert like the dense kernel -- the
gate weight COLUMNS are permuted per core so the own expert is column 0,
making the score selection a static slice. gate_b and b2 enter their
matmuls as an extra ones-row contraction step (no SBUF broadcasts).

FFN runs in bf16 (full PE rate, half the HBM bytes): gelu via the ACT Gelu
table with b1 as per-partition bias; layer-2 psum (+b2 row) is scaled by
the slot's gate score during the bf16 ACT-Copy eviction.

Combine: per-expert outputs [C, D] bf16 are AllGathered to [8C, D]; core c
owns tokens [128c, 128c+128) and pulls its 2 scaled contributions per token
with a SWDGE dma_gather (idx computed on host, int16, wrapped in 16
partitions and replicated x8), adds them, LayerNorms, and stores its
128-token shard. Host reassembles the 8 shards.

PE p-state: a short chain of junk warm-up matmuls keeps PE busy during the
initial DMA so the real matmuls run at the ramped 2.4 GHz clock.
"""

import numpy as np
import ml_dtypes

import concourse.bass as bass
import concourse.bacc as bacc
import concourse.tile as tile
from concourse import mybir
from concourse import bass_utils
from concourse import library_config

E, D, H, K, N = 8, 512, 2048, 2, 1024
P = 128
KC = D // P      # 4  contraction chunks over d_model
HC = H // P      # 16 chunks over d_hidden
EPS = 1e-5
NEG_BIG = -1e30
RSQRT2 = 0.7071067811865476

F32 = mybir.dt.float32
BF16 = mybir.dt.bfloat16
I16 = mybir.dt.int16

GW = E + HC                  # 24: xg cols = [gwT(8, permuted) | b1p(16) | Xp]
NWARM = 6                    # junk matmuls to ramp the PE p-state


def _chunked(dram, kc, p=P):
    """AP view of a [kc*P, M] DRAM tensor as [P, kc, M] (partition-major)."""
    m = dram.shape[1]
    return bass.AP(tensor=dram[:, :].tensor, offset=0,
                   ap=[[m, p], [p * m, kc], [1, m]])


def _bcast(ap, p=P):
    """AP that reads `ap` (a 1-D DRAM view) replicated across p partitions."""
    return bass.AP(tensor=ap.tensor, offset=ap.offset, ap=[[0, p]] + list(ap.ap))


def build_nc(C, single_core=False):
    """Build the SPMD program for slot capacity C (multiple of 32).

    single_core=True drops the AllGather (gather reads ybuf directly) so
    TimelineSim (single-core, no collectives) can time the kernel; numerics
    differ.
    """
    NCC = (C + P - 1) // P   # slot chunks
    nc = bacc.Bacc("TRN2", target_bir_lowering=False, debug=False,
                   num_devices=1 if single_core else E)

    xp = nc.dram_tensor("xp", [D, C], F32, kind="ExternalInput")
    gpack = nc.dram_tensor("gpack", [D, GW], F32, kind="ExternalInput")
    w1b = nc.dram_tensor("w1b", [D, H], BF16, kind="ExternalInput")  # w1[e].T
    # w2b rows: [w2[e].T (H) | b2 row + zero pad (P)]
    w2b = nc.dram_tensor("w2b", [H + P, D], BF16, kind="ExternalInput")
    gbr = nc.dram_tensor("gbr", [1, E], F32, kind="ExternalInput")  # permuted
    lwb = nc.dram_tensor("lwb", [2 * D], BF16, kind="ExternalInput")
    idx = nc.dram_tensor("idx", [P, 16], I16, kind="ExternalInput")
    out = nc.dram_tensor("out", [P, D], F32, kind="ExternalOutput")

    ybuf = nc.dram_tensor("ybuf", [C, D], BF16)
    agbuf = nc.dram_tensor("agbuf", [E * C, D], BF16)

    with tile.TileContext(nc) as tc:
        with (
            tc.tile_pool(name="persist", bufs=1) as persist,
            tc.tile_pool(name="work", bufs=4) as work,
            tc.tile_pool(name="yout", bufs=3) as yout,
            tc.tile_pool(name="psg", bufs=2, space="PSUM") as psg,
            tc.tile_pool(name="ps1", bufs=3, space="PSUM") as ps1,
            tc.tile_pool(name="ps2", bufs=3, space="PSUM") as ps2,
        ):
            # ---- xp + w1b interleaved per-k: layer-1 critical path ----
            xp_sb = persist.tile([P, KC, C], F32, tag="xp")
            xp_view = _chunked(xp, KC)
            gp_sb = persist.tile([P, KC, GW], F32, tag="gp")
            xbf = persist.tile([P, KC, C], BF16, tag="xbf")
            gbr_sb = persist.tile([P, E], F32, tag="gbr")
            nc.scalar.dma_start(out=gbr_sb[0:1, :], in_=gbr[:, :])
            idx_sb = persist.tile([P, 16], I16, tag="idx")
            nc.scalar.dma_start(out=idx_sb, in_=idx[:, :])

            # ---- warm-up junk matmuls: ramp PE while DMAs stream in ----
            jl = persist.tile([P, P], BF16, tag="jl")
            nc.vector.memset(jl, 0.0)
            jw = persist.tile([P, D], BF16, tag="jw")
            nc.vector.memset(jw, 0.0)

            def warm(n, base=[0]):
                for _ in range(n):
                    pw = ps2.tile([P, D], F32, tag="ps2",
                                  name=f"warm{base[0]}")
                    base[0] += 1
                    nc.tensor.matmul(pw, lhsT=jl, rhs=jw, start=True,
                                     stop=True)

            warm(NWARM)
            # single ACT table set for the whole kernel (erf+sigmoid+copy):
            # pull it in early, off the critical path
            eps_sb = persist.tile([P, 1], F32, tag="eps")
            nc.vector.memset(eps_sb, EPS)
            wa = persist.tile([P, 1], F32, tag="wa")
            nc.scalar.activation(wa, eps_sb, mybir.ActivationFunctionType.Erf)

            # ones rows for the bias-row matmul trick
            ones_f = persist.tile([P, C], F32, tag="ones_f")
            nc.vector.memset(ones_f[0:1, :], 1.0)
            ones_b = persist.tile([P, C], BF16, tag="ones_b")
            nc.vector.memset(ones_b[0:1, :], 1.0)

            # ---- weights: w1b/w2b interleaved to match consumption pace,
            # alternating the two HWDGE queues (SP / ACT) ----
            w1b_sb = persist.tile([P, KC, H], BF16, tag="w1b")
            w1b_view = _chunked(w1b, KC)
            w2b_sb = persist.tile([P, HC + 1, D], BF16, tag="w2b")
            w2b_view = _chunked(w2b, HC + 1)
            dmae = [nc.sync, nc.scalar]

            def w1load(k, lo, hi):
                nc.sync.dma_start(out=w1b_sb[:, k:k + 1, lo:hi],
                                  in_=w1b_view[:, k:k + 1, lo:hi])

            def w2load(lo, hi):
                nc.sync.dma_start(out=w2b_sb[:, lo:hi, :],
                                  in_=w2b_view[:, lo:hi, :])

            # consumption order on the SP queue (the ACT queue carries only
            # pre-compute loads; it must stay clear of the erf stream).
            # b2 row via ACT queue early: it seeds the psum accumulators.
            nc.scalar.dma_start(out=w2b_sb[:, 16:17, :],
                                in_=w2b_view[:, 16:17, :])
            for k in range(KC):
                nc.sync.dma_start(out=xp_sb[:, k:k + 1, :],
                                  in_=xp_view[:, k:k + 1, :])
                nc.vector.tensor_copy(out=xbf[:, k, :], in_=xp_sb[:, k, :])
                w1load(k, 0, H // 2)
            nc.sync.dma_start(out=gp_sb, in_=_chunked(gpack, KC))
            w2load(0, 2)
            w2load(2, 4)
            w1load(0, H // 2, H)
            w2load(4, 6)
            w1load(1, H // 2, H)
            w2load(6, 8)
            w1load(2, H // 2, H)
            w2load(8, 10)
            w1load(3, H // 2, H)
            w2load(10, 12)
            w2load(12, 14)
            w2load(14, 16)
            lwb_sb = persist.tile([P, 2 * D], BF16, tag="lwb")
            nc.sync.dma_start(out=lwb_sb, in_=_bcast(lwb[:]))

            nc.gpsimd.load_library(library_config.mlp)
            gsrc = ybuf if single_core else agbuf
            g2 = persist.tile([P, 2, D], BF16, tag="g2")

            lnw_sb = lwb_sb[:, 0:D]
            lnb_sb = lwb_sb[:, D:2 * D]
            b1p_sb = gp_sb[:, 0, E:E + HC]                  # [P, 16] f32
            b1h_sb = persist.tile([P, HC], F32, tag="b1h")  # b1 / sqrt(2)
            nc.vector.tensor_scalar(
                out=b1h_sb, in0=b1p_sb, scalar1=RSQRT2, scalar2=None,
                op0=mybir.AluOpType.mult,
            )

            # ---- gate: exact-f32 logits for all C slots (col 0 = own) ----
            La = persist.tile([P, NCC, E], F32, tag="La")

            def gate_mms():
              for cc in range(NCC):
                cw = min(P, C - cc * P)
                pg = psg.tile([P, E], F32, tag="psg")
                for k in range(KC):
                    nc.tensor.matmul(
                        pg[0:cw],
                        lhsT=xp_sb[:, k, cc * P:cc * P + cw],
                        rhs=gp_sb[:, k, 0:E],
                        start=(k == 0),
                        stop=False,
                    )
                nc.tensor.matmul(
                    pg[0:cw],
                    lhsT=ones_f[0:1, cc * P:cc * P + cw],
                    rhs=gbr_sb[0:1, :],
                    start=False,
                    stop=True,
                )
                nc.vector.tensor_copy(out=La[:, cc, :], in_=pg)

            def gate_chain():
                """Per-slot gate score of OWN expert (column 0), [P,NCC]."""
                X = mybir.AxisListType.X
                v1 = work.tile([P, NCC], F32, tag="v1")
                nc.vector.reduce_max(out=v1, in_=La, axis=X)
                eq1 = work.tile([P, NCC, E], F32, tag="eq1")
                nc.vector.tensor_tensor(
                    out=eq1, in0=La, in1=v1[:, :, None].to_broadcast((P, NCC, E)),
                    op=mybir.AluOpType.is_equal,
                )
                Lm = work.tile([P, NCC, E], F32, tag="Lm")
                nc.vector.scalar_tensor_tensor(
                    out=Lm, in0=eq1, scalar=NEG_BIG, in1=La,
                    op0=mybir.AluOpType.mult, op1=mybir.AluOpType.add,
                )
                v2 = work.tile([P, NCC], F32, tag="v2")
                nc.vector.reduce_max(out=v2, in_=Lm, axis=X)
                s2 = work.tile([P, NCC], F32, tag="s2")
                nc.vector.tensor_sub(s2, v2, v1)
                nc.scalar.activation(s2, s2, mybir.ActivationFunctionType.Sigmoid)
                e2s = work.tile([P, NCC], F32, tag="e2s")
                nc.vector.tensor_tensor(
                    out=e2s, in0=Lm[:, :, 0], in1=v2,
                    op=mybir.AluOpType.is_equal,
                )
                nc.vector.tensor_mul(e2s, e2s, s2)          # e2 * s2
                s1 = work.tile([P, NCC], F32, tag="s1")
                nc.vector.tensor_scalar(
                    out=s1, in0=s2, scalar1=-1.0, scalar2=1.0,
                    op0=mybir.AluOpType.mult, op1=mybir.AluOpType.add,
                )
                nc.vector.tensor_mul(s1, s1, eq1[:, :, 0])  # e1 * s1
                gcol = persist.tile([P, NCC], F32, tag="gcol")
                nc.vector.tensor_add(gcol, s1, e2s)
                return gcol

            # ---- layers 1+2 interleaved per h-chunk (L2 lags L1 by two
            # windows so the erf/ht/stt eviction chain stays off the PE
            # critical path); PE stream never drains ----
            g1 = persist.tile([P, HC, C], BF16, tag="g1")
            p2s = [ps2.tile([P, D], F32, tag="ps2", name=f"p2_{cc}")
                   for cc in range(NCC)]
            # seed each psum with the b2 ones-row (start=True) -- moves the
            # bias row off the tail and fills the early PE bubble
            for cc in range(NCC):
                cw = min(P, C - cc * P)
                nc.tensor.matmul(
                    p2s[cc][0:cw],
                    lhsT=ones_b[0:1, cc * P:cc * P + cw],
                    rhs=w2b_sb[0:1, 16, :],
                    start=True,
                    stop=False,
                )

            def l1(h, filler=0):
                p1 = ps1.tile([P, C], F32, tag="ps1")
                for k in range(KC):
                    nc.tensor.matmul(
                        p1,
                        lhsT=w1b_sb[:, k, h * P:(h + 1) * P],
                        rhs=xbf[:, k, :],
                        start=(k == 0),
                        stop=(k == KC - 1),
                    )
                    if filler and k < KC - 1:
                        warm(filler)
                # gelu = 0.5*(t)*(1+erf(t/sqrt2)), t = p1 + b1
                er = work.tile([P, C], F32, tag="er")
                nc.scalar.activation(
                    er, p1, mybir.ActivationFunctionType.Erf,
                    bias=b1h_sb[:, h:h + 1], scale=RSQRT2,
                )
                ht = work.tile([P, C], F32, tag="ht")
                nc.vector.tensor_scalar(
                    out=ht, in0=p1, scalar1=b1p_sb[:, h:h + 1], scalar2=0.5,
                    op0=mybir.AluOpType.add, op1=mybir.AluOpType.mult,
                )
                nc.vector.scalar_tensor_tensor(
                    out=g1[:, h, :], in0=er, scalar=1.0, in1=ht,
                    op0=mybir.AluOpType.add, op1=mybir.AluOpType.mult,
                )

            def l2(h):
                for cc in range(NCC):
                    cw = min(P, C - cc * P)
                    nc.tensor.matmul(
                        p2s[cc][0:cw],
                        lhsT=g1[:, h, cc * P:cc * P + cw],
                        rhs=w2b_sb[:, h, :],
                        start=False,
                        stop=(h == HC - 1),
                    )

            gcol = None
            for h in range(HC):
                l1(h)
                if h == 1:
                    gate_mms()
                if 2 <= h <= 13:
                    l2(h - 2)
                if h == 3:
                    gcol = gate_chain()
            for h in range(HC - 4, HC):
                l2(h)

            # ---- scaled bf16 eviction (ACT/DVE alternate) ----
            for cc in range(NCC):
                cw = min(P, C - cc * P)
                yb = yout.tile([P, D], BF16, tag="yb")
                if cc == 1:
                    nc.vector.tensor_scalar(
                        out=yb[0:cw], in0=p2s[cc][0:cw],
                        scalar1=gcol[0:cw, cc:cc + 1], scalar2=None,
                        op0=mybir.AluOpType.mult,
                    )
                else:
                    nc.scalar.activation(
                        yb[0:cw], p2s[cc][0:cw],
                        mybir.ActivationFunctionType.Copy,
                        scale=gcol[0:cw, cc:cc + 1],
                    )
                nc.sync.dma_start(out=ybuf[cc * P:cc * P + cw, :],
                                  in_=yb[0:cw])

            # ---- exchange + owner-side combine ----
            if not single_core:
                nc.gpsimd.collective_compute(
                    "AllGather",
                    mybir.AluOpType.bypass,
                    replica_groups=[list(range(E))],
                    ins=[ybuf[:, :].opt()],
                    outs=[agbuf[:, :].opt()],
                )
            nc.gpsimd.dma_gather(
                out_ap=g2, in_ap=gsrc[:, :], idxs_ap=idx_sb[:, :],
                num_idxs=2 * P, num_idxs_reg=2 * P, elem_size=D,
            )
            z = persist.tile([P, D], BF16, tag="z")
            nc.vector.tensor_add(z, g2[:, 0, :], g2[:, 1, :])

            # ---- LayerNorm + store ----
            stats = work.tile([P, 6], F32, tag="stats")
            nc.vector.bn_stats(out=stats, in_=z)
            mv = work.tile([P, 2], F32, tag="mv")
            nc.vector.bn_aggr(out=mv, in_=stats)
            # rstd via bit-hack + 2 Newton steps (no sqrt table needed)
            rstd = work.tile([P, 1], F32, tag="rstd")
            ve = work.tile([P, 1], F32, tag="ve")
            nc.vector.tensor_scalar(
                out=ve, in0=mv[:, 1:2], scalar1=float(EPS),
                scalar2=None, op0=mybir.AluOpType.add,
            )
            I32 = mybir.dt.int32
            nc.vector.tensor_scalar(
                out=rstd.bitcast(I32), in0=ve.bitcast(I32),
                scalar1=1, scalar2=None,
                op0=mybir.AluOpType.arith_shift_right,
            )
            nc.vector.tensor_scalar(
                out=rstd.bitcast(I32), in0=rstd.bitcast(I32),
                scalar1=-1, scalar2=0x5F3759DF,
                op0=mybir.AluOpType.mult, op1=mybir.AluOpType.add,
            )
            t1 = work.tile([P, 1], F32, tag="t1")
            for _ in range(2):        # y *= 1.5 - 0.5*v*y*y
                nc.vector.tensor_mul(t1, rstd, rstd)
                nc.vector.tensor_mul(t1, t1, ve)
                nc.vector.tensor_scalar(
                    out=t1, in0=t1, scalar1=-0.5, scalar2=1.5,
                    op0=mybir.AluOpType.mult, op1=mybir.AluOpType.add,
                )
                nc.vector.tensor_mul(rstd, rstd, t1)
            # final affine in column halves; each half's store overlaps the
            # other half's compute (two HWDGE queues)
            xn = work.tile([P, D], BF16, tag="xn")
            xo = work.tile([P, D], F32, tag="xo")
            HD = D // 2
            for i in range(2):
                cs = slice(i * HD, (i + 1) * HD)
                nc.vector.tensor_scalar(
                    out=xn[:, cs], in0=z[:, cs], scalar1=mv[:, 0:1],
                    scalar2=rstd,
                    op0=mybir.AluOpType.subtract, op1=mybir.AluOpType.mult,
                )
                nc.vector.tensor_mul(xn[:, cs], xn[:, cs], lnw_sb[:, cs])
                nc.vector.tensor_add(xo[:, cs], xn[:, cs], lnb_sb[:, cs])
                dmae[i].dma_start(out=out[:, i * HD:(i + 1) * HD],
                                  in_=xo[:, cs])

    nc.compile()
    return nc


_CACHE = {}


def _get_nc(C):
    if C not in _CACHE:
        _CACHE[C] = build_nc(C)
    return _CACHE[C]


def route(inp, gate_w, gate_b):
    """Host-side routing DECISION (top-2 expert ids per token); all scoring
    arithmetic is recomputed on-device in exact f32."""
    logits = inp.astype(np.float32) @ gate_w.T.astype(np.float32) + gate_b
    top2 = np.argsort(-logits, axis=1, kind="stable")[:, :K]   # [N, 2]
    return top2


def make_in_maps(inputs, C=None):
    inp = np.asarray(inputs["inp"], dtype=np.float32)
    gate_w = np.asarray(inputs["gate_w"], dtype=np.float32)
    gate_b = np.asarray(inputs["gate_b"], dtype=np.float32)
    w1 = np.asarray(inputs["w1"], dtype=np.float32)
    b1 = np.asarray(inputs["b1"], dtype=np.float32)
    w2 = np.asarray(inputs["w2"], dtype=np.float32)
    b2 = np.asarray(inputs["b2"], dtype=np.float32)
    ln_w = np.asarray(inputs["ln_w"], dtype=np.float32)
    ln_b = np.asarray(inputs["ln_b"], dtype=np.float32)

    top2 = route(inp, gate_w, gate_b)
    toks = [np.where((top2[:, 0] == e) | (top2[:, 1] == e))[0] for e in range(E)]
    maxc = max(len(t) for t in toks)
    if C is None:
        C = max(((maxc + 31) // 32) * 32, P)
    assert maxc <= C

    slot_of = np.full((E, N), -1, np.int64)
    for e in range(E):
        slot_of[e, toks[e]] = np.arange(len(toks[e]))

    xT = np.ascontiguousarray(inp.T)                      # [D, N]
    lwb = np.concatenate([ln_w, ln_b]).astype(ml_dtypes.bfloat16)

    in_maps = []
    for c in range(E):
        # permute experts so own expert is column 0 (order invariant for
        # max/2nd-max); host and device then agree on "column 0 = own".
        perm = [c] + [e for e in range(E) if e != c]
        xpv = np.zeros((D, C), np.float32)
        xpv[:, 0:len(toks[c])] = xT[:, toks[c]]
        gpv = np.zeros((D, GW), np.float32)
        gpv[:, 0:E] = gate_w.T[:, perm]
        # b1 pre-transposed into chunk 0: b1p[p, h] = b1[c][h*128+p]
        gpv[0:P, E:GW] = b1[c].reshape(HC, P).T
        w2v = np.zeros((H + P, D), np.float32)
        w2v[0:H] = w2[c].T
        w2v[H] = b2[c]
        # gather rows for owned tokens [128c, 128c+128): contribution k of
        # token t lives at row top2[t][k]*C + slot_of[top2[t][k], t]
        own = np.arange(P * c, P * (c + 1))
        rows = np.empty(2 * P, np.int64)
        for kk in range(K):
            ee = top2[own, kk]
            rows[kk * P:(kk + 1) * P] = ee * C + slot_of[ee, own]
        blk = np.zeros((16, 16), np.int16)
        blk[np.arange(2 * P) % 16, np.arange(2 * P) // 16] = \
            rows.astype(np.int16)
        in_maps.append({
            "xp": xpv,
            "gpack": gpv,
            "w1b": np.ascontiguousarray(w1[c].T).astype(ml_dtypes.bfloat16),
            "w2b": w2v.astype(ml_dtypes.bfloat16),
            "gbr": gate_b[perm].reshape(1, E).astype(np.float32),
            "lwb": lwb,
            "idx": np.tile(blk, (E, 1)),
        })
    return in_maps, C


def kernel(**inputs):
    in_maps, C = make_in_maps(inputs)
    nc = _get_nc(C)
    res = bass_utils.run_bass_kernel_spmd(nc, in_maps, core_ids=list(range(E)))
    full = np.empty((N, D), np.float32)
    for c in range(E):
        full[P * c:P * (c + 1)] = res.results[c]["out"]
    return full
